# revision 6
# baseline (speedup 1.0000x reference)
"""AMPBlock0 (BigVGAN) Trainium2 kernel: B=8 data-parallel over 8 NeuronCores.

Per core: x (512, 8192) f32 -> out (512, 8188) f32
  a1 = down1(snake1(up1(x)))       # polyphase up x2, SnakeBeta, stride-2 lowpass
  c1 = conv1d_3tap(a1) + b1
  a2 = down2(snake2(up2(c1)))
  out = conv1d_3tap(a2) + b2 + x[:, :8188]

Layout: channels on partitions (4 blocks x 128), time on free axis.
Time tiled (L=1024) with halos. bf16 storage/matmul, f32 PSUM.
Engines: PE = dense convs + 12-tap down convs (diag matmuls, PSUM-accum);
DVE = 6-tap up convs (scalar_tensor_tensor chains) + snake STT + residual evict;
ACT = cos via Sin(scale*acc + bias') with bias' = pi/2 - 2a*inv2b (since
acc = u + inv2b), plus PSUM evictions w/ per-channel bias.
"""

import sys

if "/opt/trn_rl_repo" not in sys.path:
    sys.path.insert(0, "/opt/trn_rl_repo")

import numpy as np
import ml_dtypes

import concourse.bass as bass
import concourse.bacc as bacc
import concourse.mybir as mybir
import concourse.tile as tile
from concourse.bass_utils import run_bass_kernel_spmd

BF16 = mybir.dt.bfloat16
F32 = mybir.dt.float32
AF = mybir.ActivationFunctionType
ALU = mybir.AluOpType

T = 8192
C = 512
NB = 4  # channel blocks of 128
L = 1024  # output cols per time tile
NT = T // L
PAD = 16  # host zero-pad of x on each side (time)
TOUT = T - 4  # 8188
CHUNK = 512  # matmul moving/psum chunk

# scalar-column layout in `sc` (128, NB, 64) f32, per stage offsets 0 / 28
# 0-5 we, 6-11 wo, 12-17 d_o, 18-23 d_e, 24 scaleA, 25 biasS, 26 ninv2b, 27 inv2b
# 56 bias1 (cout), 57 bias2 (cout)
S2 = 28

LAST_EXEC_NS = None
LAST_PROFILE = None


def _chunks(width):
    out = []
    c0 = 0
    while c0 < width:
        out.append((c0, min(CHUNK, width - c0)))
        c0 += CHUNK
    return out


def _upconv(nc, pools, sc_t, b, src_tile, width, wcol_base, inv2b_col, tag):
    """6-tap up conv for one phase/block: acc = sum_k src[.,k:k+W]*w[k] + inv2b.
    src_tile cols are pre-offset so tap k reads [k : k+width]. Returns acc tile
    slice (128, width) holding u + inv2b."""
    accp = pools[tag]
    acc = accp.tile([128, NB, width], BF16, tag=tag)
    nc.vector.tensor_scalar(
        acc[:, b, :], src_tile[:, b, 0:width],
        sc_t[:, b, wcol_base:wcol_base + 1], sc_t[:, b, inv2b_col:inv2b_col + 1],
        ALU.mult, ALU.add,
    )
    cur = acc
    for k in range(1, 6):
        nxt = accp.tile([128, NB, width], BF16, tag=tag)
        nc.vector.scalar_tensor_tensor(
            nxt[:, b, :], src_tile[:, b, k:k + width],
            sc_t[:, b, wcol_base + k:wcol_base + k + 1], cur[:, b, :],
            ALU.mult, ALU.add,
        )
        cur = nxt
    return cur


def _fix_multiwait(nc):
    """walrus's S3D3 compute-instruction structs encode at most ONE sync
    wait. Tile emits several on some instructions. Splice a same-engine
    Drain (multi-wait capable) before each such instruction, carrying all
    its waits."""
    vuln = {
        "InstActivation", "InstTensorScalarPtr", "InstScalarTensorTensor",
        "InstTensorTensor", "InstTensorScalar", "InstMatmult", "InstMemset",
        "InstTensorCopy", "InstCopy", "InstTensorReduce", "InstLdweights",
        "InstPool", "InstISA",
    }
    cnt = 0
    for f in nc.m.functions:
        for b in f.blocks:
            il = b.instructions
            newl = []
            for inst in il:
                si = inst.sync_info
                if (si is not None and len(si.on_wait) > 1
                        and type(inst).__name__ in vuln):
                    nd = mybir.InstDrain(name=f"I-mwfix-{cnt}", ins=[], outs=[])
                    cnt += 1
                    nd.engine = inst.engine
                    nd.sync_info = mybir.SyncInfo(
                        on_wait=list(si.on_wait), on_update=[])
                    inst.sync_info = mybir.SyncInfo(
                        on_wait=[], on_update=list(si.on_update))
                    newl.append(nd)
                newl.append(inst)
            il[:] = newl
    return cnt


def build_graph():
    nc = bacc.Bacc()
    xp_d = nc.declare_dram_parameter("xp", [128, NB, T + 2 * PAD], BF16, isOutput=False)
    w1t_d = nc.declare_dram_parameter("w1t", [128, 3, NB, NB, 128], BF16, isOutput=False)
    w2t_d = nc.declare_dram_parameter("w2t", [128, 3, NB, NB, 128], BF16, isOutput=False)
    sc_d = nc.declare_dram_parameter("sc", [128, NB, 64], F32, isOutput=False)
    diag_d = nc.declare_dram_parameter("diag", [128, 2 * NB * 12, 128], BF16, isOutput=False)
    brow_d = nc.declare_dram_parameter("brow", [1, NB, 128], BF16, isOutput=False)
    out_d = nc.declare_dram_parameter("out", [128, NB, TOUT], F32, isOutput=True)

    with tile.TileContext(nc) as tc:
        with (
            tc.tile_pool(name="const", bufs=1) as constp,
            tc.tile_pool(name="xt", bufs=2) as xtp,
            tc.tile_pool(name="acc_e", bufs=2) as accep,
            tc.tile_pool(name="acc_o", bufs=2) as accop,
            tc.tile_pool(name="cos", bufs=1) as cosp,
            tc.tile_pool(name="sE", bufs=2) as sEp,
            tc.tile_pool(name="sO", bufs=2) as sOp,
            tc.tile_pool(name="amid", bufs=2) as amidp,
            tc.tile_pool(name="c1", bufs=2) as c1p,
            tc.tile_pool(name="outt", bufs=1) as outp,
            tc.tile_pool(name="dg", bufs=2) as dgp,
            tc.tile_pool(name="dps", bufs=3, space="PSUM") as dpsp,
            tc.tile_pool(name="wps", bufs=3, space="PSUM") as wpsp,
        ):
            pools = {"acc_e": accep, "acc_o": accop}

            # resident constants
            w1t = constp.tile([128, 3, NB, NB, 128], BF16)
            nc.sync.dma_start(w1t[:], w1t_d[:])
            w2t = constp.tile([128, 3, NB, NB, 128], BF16)
            nc.sync.dma_start(w2t[:], w2t_d[:])
            sc_t = constp.tile([128, NB, 64], F32)
            nc.sync.dma_start(sc_t[:], sc_d[:])
            brow = constp.tile([1, NB, 128], BF16)
            nc.sync.dma_start(brow[:], brow_d[:])
            ones = constp.tile([1, CHUNK], BF16)
            nc.gpsimd.memset(ones[:], 1.0)


            for i in range(NT):
                t0 = i * L
                first, last = i == 0, i == NT - 1
                W1, s1 = L + 21, t0 - 8   # u1 / E1 / O1
                W2, s2 = L + 15, t0 - 6   # a1
                W3, s3 = L + 13, t0 - 5   # c1
                W4, s4 = L + 8, t0 - 3    # u2 / E2 / O2
                W5, s5 = L + 2, t0 - 1    # a2
                Wx, sx = L + 26, t0 - 10  # x

                xt = xtp.tile([128, NB, Wx], BF16)
                nc.sync.dma_start(xt[:], xp_d[:, :, sx + PAD:sx + PAD + Wx])

                # ---------------- stage 1: up + snake ----------------
                E1 = sEp.tile([128, NB, W1], BF16, tag="sE")
                O1 = sOp.tile([128, NB, W1], BF16, tag="sO")
                for b in range(NB):
                    for phase, (wbase, dst) in enumerate(((0, E1), (6, O1))):
                        tag = "acc_e" if phase == 0 else "acc_o"
                        acc = _upconv(nc, pools, sc_t, b, xt, W1, wbase, 27, tag)
                        cost = cosp.tile([128, NB, W1], BF16, tag="cos")
                        nc.scalar.activation(
                            cost[:, b, :], acc[:, b, :], AF.Sin,
                            bias=sc_t[:, b, 25:26], scale=sc_t[:, b, 24:25],
                        )
                        nc.vector.scalar_tensor_tensor(
                            dst[:, b, :], cost[:, b, :], sc_t[:, b, 26:27],
                            acc[:, b, :], ALU.mult, ALU.add,
                        )
                # E/O valid m in [1, T-2], zero outside
                if first:
                    nc.gpsimd.memset(E1[:, :, 0:1 - s1], 0.0)
                    nc.gpsimd.memset(O1[:, :, 0:1 - s1], 0.0)
                if last:
                    z = (T - 1) - s1
                    nc.gpsimd.memset(E1[:, :, z:W1], 0.0)
                    nc.gpsimd.memset(O1[:, :, z:W1], 0.0)

                # ---------------- down1 (PE diag) -> a1 ----------------
                dg1 = dgp.tile([128, NB * 12, 128], BF16, tag="dg")
                nc.sync.dma_start(dg1[:], diag_d[:, 0:NB * 12, :])
                a1 = amidp.tile([128, NB, W2], BF16, tag="amid")
                for b in range(NB):
                    for c0, n in _chunks(W2):
                        ps = wpsp.tile([128, CHUNK], F32, tag="wps")
                        for r in range(6):
                            nc.tensor.matmul(
                                ps[:, :n], dg1[:, b * 12 + r, :],
                                O1[:, b, c0 + r:c0 + r + n],
                                start=(r == 0), stop=False,
                            )
                        for r in range(6):
                            nc.tensor.matmul(
                                ps[:, :n], dg1[:, b * 12 + 6 + r, :],
                                E1[:, b, c0 + r + 1:c0 + r + 1 + n],
                                start=False, stop=(r == 5),
                            )
                        nc.scalar.copy(a1[:, b, c0:c0 + n], ps[:, :n])
                if first:
                    nc.gpsimd.memset(a1[:, :, 0:0 - s2], 0.0)
                if last:
                    z = (T - 2) - s2
                    nc.gpsimd.memset(a1[:, :, z:W2], 0.0)

                # ---------------- conv1 (PE dense) -> c1 ----------------
                c1 = c1p.tile([128, NB, W3], BF16, tag="c1")
                for o in range(NB):
                    for c0, n in _chunks(W3):
                        ps = dpsp.tile([128, CHUNK], F32, tag="dps")
                        for idx, (ib, k) in enumerate(
                            (ib, k) for ib in range(NB) for k in range(3)
                        ):
                            nc.tensor.matmul(
                                ps[:, :n], w1t[:, k, ib, o, :],
                                a1[:, ib, c0 + k:c0 + k + n],
                                start=(idx == 0), stop=(idx == 11),
                            )
                        nc.scalar.activation(
                            c1[:, o, c0:c0 + n], ps[:, :n], AF.Identity,
                            bias=sc_t[:, o, 56:57], scale=1.0,
                        )
                if first:
                    nc.gpsimd.memset(c1[:, :, 0:0 - s3], 0.0)
                if last:
                    z = (T - 2) - s3
                    nc.gpsimd.memset(c1[:, :, z:W3], 0.0)

                # ---------------- stage 2: up + snake ----------------
                E2 = sEp.tile([128, NB, W4], BF16, tag="sE")
                O2 = sOp.tile([128, NB, W4], BF16, tag="sO")
                for b in range(NB):
                    for phase, (wbase, dst) in enumerate(((S2 + 0, E2), (S2 + 6, O2))):
                        tag = "acc_e" if phase == 0 else "acc_o"
                        acc = _upconv(nc, pools, sc_t, b, c1, W4, wbase, S2 + 27, tag)
                        cost = cosp.tile([128, NB, W4], BF16, tag="cos")
                        nc.scalar.activation(
                            cost[:, b, :], acc[:, b, :], AF.Sin,
                            bias=sc_t[:, b, S2 + 25:S2 + 26], scale=sc_t[:, b, S2 + 24:S2 + 25],
                        )
                        nc.vector.scalar_tensor_tensor(
                            dst[:, b, :], cost[:, b, :], sc_t[:, b, S2 + 26:S2 + 27],
                            acc[:, b, :], ALU.mult, ALU.add,
                        )
                # E2/O2 valid m in [1, (T-2)-2], zero outside
                if first:
                    nc.gpsimd.memset(E2[:, :, 0:1 - s4], 0.0)
                    nc.gpsimd.memset(O2[:, :, 0:1 - s4], 0.0)
                if last:
                    z = (T - 3) - s4
                    nc.gpsimd.memset(E2[:, :, z:W4], 0.0)
                    nc.gpsimd.memset(O2[:, :, z:W4], 0.0)

                # ---------------- down2 (PE diag) -> a2 ----------------
                dg2 = dgp.tile([128, NB * 12, 128], BF16, tag="dg")
                nc.sync.dma_start(dg2[:], diag_d[:, NB * 12:2 * NB * 12, :])
                a2 = amidp.tile([128, NB, W5], BF16, tag="amid")
                for b in range(NB):
                    for c0, n in _chunks(W5):
                        ps = wpsp.tile([128, CHUNK], F32, tag="wps")
                        for r in range(6):
                            nc.tensor.matmul(
                                ps[:, :n], dg2[:, b * 12 + r, :],
                                O2[:, b, c0 + r:c0 + r + n],
                                start=(r == 0), stop=False,
                            )
                        for r in range(6):
                            nc.tensor.matmul(
                                ps[:, :n], dg2[:, b * 12 + 6 + r, :],
                                E2[:, b, c0 + r + 1:c0 + r + 1 + n],
                                start=False, stop=(r == 5),
                            )
                        nc.scalar.copy(a2[:, b, c0:c0 + n], ps[:, :n])
                if first:
                    nc.gpsimd.memset(a2[:, :, 0:0 - s5], 0.0)
                if last:
                    z = (T - 4) - s5
                    nc.gpsimd.memset(a2[:, :, z:W5], 0.0)

                # ---------------- conv2 + bias + residual -> out ----------------
                Lo = min(L, TOUT - t0)
                outt = outp.tile([128, NB, L], F32, tag="outt")
                for o in range(NB):
                    c0 = 0
                    while c0 < Lo:
                        n = min(CHUNK, Lo - c0)
                        ps = dpsp.tile([128, CHUNK], F32, tag="dps")
                        for idx, (ib, k) in enumerate(
                            (ib, k) for ib in range(NB) for k in range(3)
                        ):
                            nc.tensor.matmul(
                                ps[:, :n], w2t[:, k, ib, o, :],
                                a2[:, ib, c0 + k:c0 + k + n],
                                start=(idx == 0), stop=False,
                            )
                        nc.tensor.matmul(
                            ps[:, :n], brow[0:1, o, :], ones[0:1, :n],
                            start=False, stop=True,
                        )
                        nc.vector.scalar_tensor_tensor(
                            outt[:, o, c0:c0 + n], xt[:, o, 10 + c0:10 + c0 + n],
                            1.0, ps[:, :n], ALU.mult, ALU.add,
                        )
                        c0 += n
                nc.sync.dma_start(out_d[:, :, t0:t0 + Lo], outt[:, :, 0:Lo])
    nc.finalize()
    return nc


def _prep_host(x, up_w1, down_w1, alpha1, beta1, up_w2, down_w2, alpha2, beta2,
               c1_w, c1_b, c2_w, c2_b):
    bf = ml_dtypes.bfloat16
    B = x.shape[0]

    def dense_wt(w):
        # (128, 3, NB_in... ) W1T[ci, k, i, o, co] = w[o*128+co, i*128+ci, k]
        wr = w.reshape(NB, 128, NB, 128, 3)  # [o, co, i, ci, k]
        return np.ascontiguousarray(wr.transpose(3, 4, 2, 0, 1)[0] if False else
                                    wr.transpose(3, 4, 2, 0, 1))

    # simpler: build explicitly
    def dense_wt2(w):
        out = np.empty((128, 3, NB, NB, 128), np.float32)
        wr = w.reshape(NB, 128, NB, 128, 3)  # o, co, i, ci, k
        # out[ci, k, i, o, co]
        out[:] = wr.transpose(3, 4, 2, 0, 1)  # (ci, k, i, o, co)
        return out.astype(bf)

    w1t = dense_wt2(c1_w)
    w2t = dense_wt2(c2_w)

    sc = np.zeros((128, NB, 64), np.float32)
    cidx = np.arange(C)
    for s, (up_w, down_w, alpha, beta) in enumerate(
        ((up_w1, down_w1, alpha1, beta1), (up_w2, down_w2, alpha2, beta2))
    ):
        off = s * S2
        a2v = 2.0 * np.exp(alpha)
        inv2b = 1.0 / (2.0 * np.exp(beta) + 1e-9)
        for b in range(NB):
            cs = cidx[b * 128:(b + 1) * 128]
            for k in range(6):
                sc[:, b, off + k] = up_w[2 * cs, k]
                sc[:, b, off + 6 + k] = up_w[2 * cs + 1, k]
                sc[:, b, off + 12 + k] = down_w[cs, 2 * k]      # d_o
                sc[:, b, off + 18 + k] = down_w[cs, 2 * k + 1]  # d_e
            sc[:, b, off + 24] = a2v[cs]
            sc[:, b, off + 25] = np.pi / 2 - a2v[cs] * inv2b[cs]
            sc[:, b, off + 26] = -inv2b[cs]
            sc[:, b, off + 27] = inv2b[cs]
    for b in range(NB):
        cs = cidx[b * 128:(b + 1) * 128]
        sc[:, b, 56] = c1_b[cs]
        sc[:, b, 57] = c2_b[cs]

    diag = np.zeros((128, 2 * NB * 12, 128), np.float32)
    for s, down_w in enumerate((down_w1, down_w2)):
        for b in range(NB):
            cs = cidx[b * 128:(b + 1) * 128]
            for r in range(6):
                i0 = s * NB * 12 + b * 12
                diag[np.arange(128), i0 + r, np.arange(128)] = down_w[cs, 2 * r]
                diag[np.arange(128), i0 + 6 + r, np.arange(128)] = down_w[cs, 2 * r + 1]
    diag = diag.astype(bf)

    brow = np.zeros((1, NB, 128), np.float32)
    for o in range(NB):
        brow[0, o, :] = c2_b[o * 128:(o + 1) * 128]
    brow = brow.astype(bf)

    in_maps = []
    for bi in range(B):
        xpad = np.zeros((C, T + 2 * PAD), np.float32)
        xpad[:, PAD:PAD + T] = x[bi]
        xp = np.ascontiguousarray(
            xpad.reshape(NB, 128, T + 2 * PAD).transpose(1, 0, 2)
        ).astype(bf)
        in_maps.append({
            "xp": xp, "w1t": w1t, "w2t": w2t, "sc": sc.astype(np.float32),
            "diag": diag, "brow": brow,
        })
    return in_maps


_NC_CACHE = None


def _install_profile_hook():
    """Recreate antenv.axon_hooks (absent in this image) so
    run_bass_kernel_spmd(trace=True) can NTFF-profile via libaxon."""
    import types

    try:
        from antenv.axon_hooks import get_axon_ntff_profile_hook  # noqa: F401
        return
    except ImportError:
        pass
    try:
        import antenv
        mod = types.ModuleType("antenv.axon_hooks")
        _state = {"hook": None}
        mod.set_axon_ntff_profile_hook = lambda h: _state.__setitem__("hook", h)
        mod.get_axon_ntff_profile_hook = lambda: _state["hook"]
        sys.modules["antenv.axon_hooks"] = mod
        antenv.axon_hooks = mod
        if "/root/.axon_site" not in sys.path:
            sys.path.insert(0, "/root/.axon_site")
        from trn_agent_boot.trn_boot import _ntff_profile_via_ctypes
        mod.set_axon_ntff_profile_hook(
            _ntff_profile_via_ctypes("/opt/axon/libaxon_pjrt.so"))
    except Exception as e:  # degrade silently; tracing is optional
        print(f"profile hook install failed: {e}")


def kernel(**inputs):
    global _NC_CACHE, LAST_EXEC_NS, LAST_PROFILE
    import os

    args = {k: np.asarray(v) for k, v in inputs.items()}
    in_maps = _prep_host(**args)
    if _NC_CACHE is None:
        _NC_CACHE = build_graph()
    nc = _NC_CACHE
    trace = bool(os.environ.get("KERNEL_TRACE"))
    kw = {}
    if trace:
        _install_profile_hook()
        kw["tmpdir"] = os.environ.get("KERNEL_TRACE_DIR", "/tmp/ktrace")
        os.makedirs(kw["tmpdir"], exist_ok=True)
    res = run_bass_kernel_spmd(
        nc, in_maps, core_ids=list(range(8)), trace=trace, **kw,
    )
    LAST_EXEC_NS = res.exec_time_ns
    LAST_PROFILE = res.profile_json
    B = len(in_maps)
    out = np.empty((B, C, TOUT), np.float32)
    for bi in range(B):
        o = res.results[bi]["out"]  # (128, NB, TOUT)
        out[bi] = o.transpose(1, 0, 2).reshape(C, TOUT)
    return out
